# revision 4
# baseline (speedup 1.0000x reference)
"""Trainium2 Bass kernel for nn_PointPairwiseRelation3.

Reference computation (B=2, N=256, N1=N2=128, C=Co=16):
    h1[b,n,j,k,:] = relu(x[b,n]@Wa + (x1[b,j]-x[b,n])@Wb + (x2[b,k]-x[b,n])@Wc + b1)
    h2 = relu(h1 @ W2 + b2)
    out[b,n,:8]  = max_{j,k}  h2[..., :8]
    out[b,n,8:]  = mean_{j,k} h2[..., 8:]

Key decomposition: the first-layer pre-activation is u[b,n] + p[b,j] + q[b,k]
with u = x@(Wa-Wb-Wc)+b1, p = x1@Wb, q = x2@Wc (tiny host-side matmuls).
S[b] = p ⊕ q (outer sum over (j,k)) is shared by every query point n of
batch b.

Device layout (per core; 8 cores, each handles 64 query points of one b):
  - partitions = 8 query points (g') x 16 channels (c), i.e. part = 16*g'+c
  - free dim   = the 16384 (j,k) pairs, processed in 2048-wide chunks
  - 8 groups g of 8 query points each; per (g, chunk):
      DVE : T = relu(S + u_g)            tensor_scalar, bf16 4x mode
      PE  : M = BDt.T @ T                BD = kron(I8, W2), PSUM fp32
      ACT : R = relu(M + b2), sums += R  activation w/ accum_out (mean pool)
      DVE : rmax = max(rmax, R)          tensor_max fold (max pool)
  - group end: reduce sums/rmax -> [128,1] staged to output tile
Host applies the final gather: max channels c<8 from rmax, mean c>=8 from
sums/16384.
"""

import numpy as np
import ml_dtypes

import concourse.bass as bass
import concourse.bacc as bacc
import concourse.mybir as mybir
from concourse.tile import TileContext
from concourse import bass_utils

# Problem constants (hardcoded; kernel.py must be self-contained).
B = 2
N = 256
N1 = 128
N2 = 128
C = 16
CO = 16
NCORES = 8
CORES_PER_B = NCORES // B          # 4 cores per batch element
N_PER_CORE = N // CORES_PER_B      # 64 query points per core
NG = N_PER_CORE // 8               # 8 groups of 8 query points
PAIRS = N1 * N2                    # 16384
CHUNK = 2048                       # free-dim chunk (4 PSUM banks)
NCHUNK = PAIRS // CHUNK            # 8
FOLDW = 1024                       # running-max fold width

BF16 = mybir.dt.bfloat16
F32 = mybir.dt.float32

_CACHE = {}


def _build_bass():
    nc = bacc.Bacc("TRN2", target_bir_lowering=False)

    S_d = nc.dram_tensor("S", (128, PAIRS), BF16, kind="ExternalInput")
    U_d = nc.dram_tensor("U", (128, NG), F32, kind="ExternalInput")
    BD_d = nc.dram_tensor("BD", (128, 128), BF16, kind="ExternalInput")
    B2_d = nc.dram_tensor("B2", (128, 1), F32, kind="ExternalInput")
    OUTS_d = nc.dram_tensor("OUTS", (128, NG), F32, kind="ExternalOutput")
    OUTM_d = nc.dram_tensor("OUTM", (128, NG), F32, kind="ExternalOutput")

    add = mybir.AluOpType.add
    amax = mybir.AluOpType.max
    relu = mybir.ActivationFunctionType.Relu
    AX = mybir.AxisListType.X

    with TileContext(nc) as tc:
        with (
            tc.tile_pool(name="const", bufs=1) as cpool,
            tc.tile_pool(name="work", bufs=3) as wpool,
            tc.tile_pool(name="psum", bufs=2, space="PSUM") as ppool,
            tc.tile_pool(name="red", bufs=2) as rpool,
        ):
            # Constants / inputs. S is loaded in CHUNK-wide sections so the
            # first groups' compute can start before the whole 4MB arrives.
            S_sec = []
            for s in range(NCHUNK):
                t = cpool.tile([128, CHUNK], BF16, tag=f"S{s}")
                nc.sync.dma_start(out=t, in_=S_d[:, s * CHUNK:(s + 1) * CHUNK])
                S_sec.append(t)
            U_t = cpool.tile([128, NG], F32, tag="U")
            nc.sync.dma_start(out=U_t, in_=U_d[:, :])
            BD_t = cpool.tile([128, 128], BF16, tag="BD")
            nc.sync.dma_start(out=BD_t, in_=BD_d[:, :])
            B2_t = cpool.tile([128, 1], F32, tag="B2")
            nc.sync.dma_start(out=B2_t, in_=B2_d[:, :])
            OUTS_t = cpool.tile([128, NG], F32, tag="OUTS")
            OUTM_t = cpool.tile([128, NG], F32, tag="OUTM")

            for g in range(NG):
                sums = rpool.tile([128, NCHUNK], F32, tag="sums")
                rmax = rpool.tile([128, FOLDW], BF16, tag="rmax")
                for s in range(NCHUNK):
                    T_t = wpool.tile([128, CHUNK], BF16, tag="T")
                    nc.vector.tensor_scalar(
                        out=T_t,
                        in0=S_sec[s],
                        scalar1=U_t[:, g:g + 1],
                        scalar2=0.0,
                        op0=add,
                        op1=amax,
                    )
                    M_t = ppool.tile([128, CHUNK], F32, tag="M")
                    for i in range(CHUNK // 512):
                        nc.tensor.matmul(
                            M_t[:, i * 512:(i + 1) * 512],
                            BD_t,
                            T_t[:, i * 512:(i + 1) * 512],
                            start=True,
                            stop=True,
                        )
                    R_t = wpool.tile([128, CHUNK], BF16, tag="R")
                    nc.scalar.activation(
                        out=R_t,
                        in_=M_t,
                        func=relu,
                        bias=B2_t[:, 0:1],
                        accum_out=sums[:, s:s + 1],
                    )
                    # Running max fold at FOLDW width.
                    nf = CHUNK // FOLDW
                    if s == 0:
                        nc.vector.tensor_tensor(
                            out=rmax, in0=R_t[:, 0:FOLDW],
                            in1=R_t[:, FOLDW:2 * FOLDW], op=amax,
                        )
                        rest = range(2, nf)
                    else:
                        rest = range(nf)
                    for i in rest:
                        nc.vector.tensor_tensor(
                            out=rmax, in0=rmax,
                            in1=R_t[:, i * FOLDW:(i + 1) * FOLDW], op=amax,
                        )
                nc.vector.tensor_reduce(
                    out=OUTS_t[:, g:g + 1], in_=sums, axis=AX, op=add)
                nc.vector.tensor_reduce(
                    out=OUTM_t[:, g:g + 1], in_=rmax, axis=AX, op=amax)

            nc.sync.dma_start(out=OUTS_d[:, :], in_=OUTS_t)
            nc.sync.dma_start(out=OUTM_d[:, :], in_=OUTM_t)

    nc.finalize()
    return nc


def _prep_inputs(inputs):
    """Host-side prep: tiny projections + per-core input maps."""
    x = np.asarray(inputs["x"], np.float32)
    x1 = np.asarray(inputs["x1"], np.float32)
    x2 = np.asarray(inputs["x2"], np.float32)
    W1 = np.asarray(inputs["W1"], np.float32)
    b1 = np.asarray(inputs["b1"], np.float32)
    W2 = np.asarray(inputs["W2"], np.float32)
    b2 = np.asarray(inputs["b2"], np.float32)

    Wa, Wb, Wc = W1[:C], W1[C:2 * C], W1[2 * C:]
    u = x @ (Wa - Wb - Wc) + b1          # [B, N, CO]
    p = x1 @ Wb                          # [B, N1, CO]
    q = x2 @ Wc                          # [B, N2, CO]

    BD = np.kron(np.eye(8, dtype=np.float32), W2).astype(ml_dtypes.bfloat16)
    B2col = np.tile(b2, 8).reshape(128, 1).astype(np.float32)

    in_maps = []
    for core in range(NCORES):
        b = core // CORES_PER_B
        n0 = (core % CORES_PER_B) * N_PER_CORE
        # S16[c, j*N2+k] = p[b,j,c] + q[b,k,c]; replicate over 8 groups.
        S16 = (p[b][:, None, :] + q[b][None, :, :]).transpose(2, 0, 1)
        S16 = S16.reshape(CO, PAIRS)
        S128 = np.tile(S16, (8, 1)).astype(ml_dtypes.bfloat16)
        # U[16*gp+c, g] = u[b, n0+8*g+gp, c]
        U = u[b, n0:n0 + N_PER_CORE].reshape(NG, 8, CO)
        U = U.transpose(1, 2, 0).reshape(128, NG).astype(np.float32)
        U = np.ascontiguousarray(U)
        in_maps.append({"S": S128, "U": U, "BD": BD, "B2": B2col})
    return in_maps


def _gather_output(results):
    """Combine per-core OUTM/OUTS into the full [B, N, CO] output."""
    out = np.zeros((B, N, CO), np.float32)
    for core, res in enumerate(results):
        b = core // CORES_PER_B
        n0 = (core % CORES_PER_B) * N_PER_CORE
        m = np.asarray(res["OUTM"], np.float32).reshape(8, CO, NG)
        s = np.asarray(res["OUTS"], np.float32).reshape(8, CO, NG)
        m = m.transpose(2, 0, 1).reshape(N_PER_CORE, CO)   # [n-n0, c]
        s = s.transpose(2, 0, 1).reshape(N_PER_CORE, CO)
        out[b, n0:n0 + N_PER_CORE, :CO // 2] = m[:, :CO // 2]
        out[b, n0:n0 + N_PER_CORE, CO // 2:] = s[:, CO // 2:] / float(PAIRS)
    return out


def _run(inputs, trace=False, **spmd_kwargs):
    if "nc" not in _CACHE:
        _CACHE["nc"] = _build_bass()
    nc = _CACHE["nc"]
    in_maps = _prep_inputs(inputs)
    res = bass_utils.run_bass_kernel_spmd(
        nc, in_maps, core_ids=list(range(NCORES)), trace=trace, **spmd_kwargs)
    return _gather_output(res.results), res


def kernel(**inputs):
    out, _ = _run(inputs)
    return out


# revision 8
# speedup vs baseline: 1.0433x; 1.0433x over previous
"""Trainium2 Bass kernel for nn_PointPairwiseRelation3.

Reference computation (B=2, N=256, N1=N2=128, C=Co=16):
    h1[b,n,j,k,:] = relu(x[b,n]@Wa + (x1[b,j]-x[b,n])@Wb + (x2[b,k]-x[b,n])@Wc + b1)
    h2 = relu(h1 @ W2 + b2)
    out[b,n,:8]  = max_{j,k}  h2[..., :8]
    out[b,n,8:]  = mean_{j,k} h2[..., 8:]

Key decomposition: the first-layer pre-activation is u[b,n] + p[b,j] + q[b,k]
with u = x@(Wa-Wb-Wc)+b1, p = x1@Wb, q = x2@Wc (tiny host-side matmuls).
S[b] = p (+) q (outer sum over all (j,k) pairs) is shared by every query
point n of batch b; it is precomputed on the host, replicated over the 8
partition groups, and DMA'd in.

Device layout (per core; 8 cores, each handles 64 query points of one b):
  - partitions of T = 8 query points (gp) x 16 channels (c): part = 16*gp+c
  - free dim = the 16384 (j,k) pairs, in 2048-wide T tiles (4 chunks of 512)
  - 8 groups g of 8 query points each.

Channel-split second layer (mean-pooled channels are o>=8, max-pooled o<8):
two M=64 stationary matrices BDS = kron(I8, W2[:, 8:]) and
BDM = kron(I8, W2[:, :8]). For each pair of 512-col chunks (A, B):
    PSUM_s[ 0: 64, fd] = BDS.T @ T_A     PSUM_m[ 0: 64, fd] = BDM.T @ T_A
    PSUM_s[64:128, fd] = BDS.T @ T_B     PSUM_m[64:128, fd] = BDM.T @ T_B
so one [128, 1024] PSUM tile carries TWO chunks (2048 cols) of sum-half
(or max-half) data -> each pooling pass touches half the elements:
  ACT : activation(relu, PSUM_s + b2s, accum_out=sums slot)   (mean pool)
  DVE : tensor_tensor_reduce(max over PSUM_m, chained via scalar init)
        (max pool, fp32; relu+b2 deferred to the host: both monotonic)
  DVE : T = relu(S + u_g) via tensor_scalar (bf16 4x mode); N_ACT_T of the
        8 per-group T tiles are produced on ACT (activation bias=u) to
        balance engine load.
Host gather: mean = (sum_halfA + sum_halfB)/16384, max =
relu(max(max_halfA, max_halfB) + b2).
"""

import numpy as np
import ml_dtypes

import concourse.bass as bass
import concourse.bacc as bacc
import concourse.mybir as mybir
from concourse.tile import TileContext
from concourse import bass_utils

# Problem constants (hardcoded; kernel.py must be self-contained).
B = 2
N = 256
N1 = 128
N2 = 128
C = 16
CO = 16
NCORES = 8
CORES_PER_B = NCORES // B          # 4 cores per batch element
N_PER_CORE = N // CORES_PER_B      # 64 query points per core
NG = N_PER_CORE // 8               # 8 groups of 8 query points
PAIRS = N1 * N2                    # 16384
TCH = 2048                         # T tile width (4 chunks of 512)
NT = PAIRS // TCH                  # 8 T tiles per group
PFD = TCH // 2                     # PSUM tile free dim (2 chunks stacked)
N_ACT_T = 2                        # how many of the NT T-tiles ACT produces

BF16 = mybir.dt.bfloat16
F32 = mybir.dt.float32

_CACHE = {}


def _build_bass():
    nc = bacc.Bacc("TRN2", target_bir_lowering=False)

    S_d = nc.dram_tensor("S", (128, PAIRS), BF16, kind="ExternalInput")
    U_d = nc.dram_tensor("U", (128, NG), F32, kind="ExternalInput")
    BDS_d = nc.dram_tensor("BDS", (128, 64), BF16, kind="ExternalInput")
    BDM_d = nc.dram_tensor("BDM", (128, 64), BF16, kind="ExternalInput")
    B2S_d = nc.dram_tensor("B2S", (128, 1), F32, kind="ExternalInput")
    OUTS_d = nc.dram_tensor("OUTS", (128, NG), F32, kind="ExternalOutput")
    OUTM_d = nc.dram_tensor("OUTM", (128, NG), F32, kind="ExternalOutput")

    add = mybir.AluOpType.add
    amax = mybir.AluOpType.max
    relu = mybir.ActivationFunctionType.Relu
    AX = mybir.AxisListType.X

    with TileContext(nc) as tc:
        with (
            tc.tile_pool(name="const", bufs=1) as cpool,
            tc.tile_pool(name="work", bufs=3) as wpool,
            tc.tile_pool(name="psum", bufs=2, space="PSUM") as ppool,
            tc.tile_pool(name="red", bufs=2) as rpool,
        ):
            # Constants / inputs. S is loaded in TCH-wide sections so the
            # first groups' compute can start before the whole 4MB arrives.
            S_sec = []
            for s in range(NT):
                t = cpool.tile([128, TCH], BF16, tag=f"S{s}")
                nc.sync.dma_start(out=t, in_=S_d[:, s * TCH:(s + 1) * TCH])
                S_sec.append(t)
            U_t = cpool.tile([128, NG], F32, tag="U")
            nc.sync.dma_start(out=U_t, in_=U_d[:, :])
            BDS_t = cpool.tile([128, 64], BF16, tag="BDS")
            nc.sync.dma_start(out=BDS_t, in_=BDS_d[:, :])
            BDM_t = cpool.tile([128, 64], BF16, tag="BDM")
            nc.sync.dma_start(out=BDM_t, in_=BDM_d[:, :])
            B2S_t = cpool.tile([128, 1], F32, tag="B2S")
            nc.sync.dma_start(out=B2S_t, in_=B2S_d[:, :])
            OUTS_t = cpool.tile([128, NG], F32, tag="OUTS")
            OUTM_t = cpool.tile([128, NG], F32, tag="OUTM")

            for g in range(NG):
                sums = rpool.tile([128, NT], F32, tag="sums")
                mslots = rpool.tile([128, NT], F32, tag="mslots")
                for t in range(NT):
                    T_t = wpool.tile([128, TCH], BF16, tag="T")
                    if t < N_ACT_T:
                        # ACT-produced T: relu(1.0 * S + u) via activation.
                        nc.scalar.activation(
                            out=T_t, in_=S_sec[t], func=relu,
                            bias=U_t[:, g:g + 1],
                        )
                    else:
                        nc.vector.tensor_scalar(
                            out=T_t, in0=S_sec[t],
                            scalar1=U_t[:, g:g + 1], scalar2=0.0,
                            op0=add, op1=amax,
                        )
                    Ms = ppool.tile([128, PFD], F32, tag="Ms")
                    Mm = ppool.tile([128, PFD], F32, tag="Mm")
                    for pr in range(TCH // 1024):
                        fa = 1024 * pr
                        fb = fa + 512
                        fd = 512 * pr
                        nc.tensor.matmul(
                            Ms[0:64, fd:fd + 512], BDS_t,
                            T_t[:, fa:fa + 512], start=True, stop=True)
                        nc.tensor.matmul(
                            Ms[64:128, fd:fd + 512], BDS_t,
                            T_t[:, fb:fb + 512], start=True, stop=True)
                        nc.tensor.matmul(
                            Mm[0:64, fd:fd + 512], BDM_t,
                            T_t[:, fa:fa + 512], start=True, stop=True)
                        nc.tensor.matmul(
                            Mm[64:128, fd:fd + 512], BDM_t,
                            T_t[:, fb:fb + 512], start=True, stop=True)
                    # Mean-pool half: relu + bias + accumulate on ACT.
                    Js = wpool.tile([128, PFD], BF16, tag="Js")
                    nc.scalar.activation(
                        out=Js, in_=Ms, func=relu, bias=B2S_t[:, 0:1],
                        accum_out=sums[:, t:t + 1],
                    )
                    # Max-pool half: per-tile max-reduce off PSUM (fp32).
                    nc.vector.tensor_reduce(
                        out=mslots[:, t:t + 1], in_=Mm, axis=AX, op=amax)
                nc.vector.tensor_reduce(
                    out=OUTS_t[:, g:g + 1], in_=sums, axis=AX, op=add)
                nc.vector.tensor_reduce(
                    out=OUTM_t[:, g:g + 1], in_=mslots, axis=AX, op=amax)

            nc.sync.dma_start(out=OUTS_d[:, :], in_=OUTS_t)
            nc.sync.dma_start(out=OUTM_d[:, :], in_=OUTM_t)

    nc.finalize()
    return nc


def _prep_inputs(inputs):
    """Host-side prep: tiny projections + per-core input maps."""
    x = np.asarray(inputs["x"], np.float32)
    x1 = np.asarray(inputs["x1"], np.float32)
    x2 = np.asarray(inputs["x2"], np.float32)
    W1 = np.asarray(inputs["W1"], np.float32)
    b1 = np.asarray(inputs["b1"], np.float32)
    W2 = np.asarray(inputs["W2"], np.float32)
    b2 = np.asarray(inputs["b2"], np.float32)

    Wa, Wb, Wc = W1[:C], W1[C:2 * C], W1[2 * C:]
    u = x @ (Wa - Wb - Wc) + b1          # [B, N, CO]
    p = x1 @ Wb                          # [B, N1, CO]
    q = x2 @ Wc                          # [B, N2, CO]

    eye8 = np.eye(8, dtype=np.float32)
    BDS = np.kron(eye8, W2[:, CO // 2:]).astype(ml_dtypes.bfloat16)
    BDM = np.kron(eye8, W2[:, :CO // 2]).astype(ml_dtypes.bfloat16)
    B2S = np.tile(b2[CO // 2:], 16).reshape(128, 1).astype(np.float32)

    in_maps = []
    for core in range(NCORES):
        b = core // CORES_PER_B
        n0 = (core % CORES_PER_B) * N_PER_CORE
        # S16[c, j*N2+k] = p[b,j,c] + q[b,k,c]; replicate over 8 groups.
        S16 = (p[b][:, None, :] + q[b][None, :, :]).transpose(2, 0, 1)
        S16 = S16.reshape(CO, PAIRS)
        S128 = np.tile(S16, (8, 1)).astype(ml_dtypes.bfloat16)
        # U[16*gp+c, g] = u[b, n0+8*g+gp, c]
        U = u[b, n0:n0 + N_PER_CORE].reshape(NG, 8, CO)
        U = np.ascontiguousarray(U.transpose(1, 2, 0).reshape(128, NG),
                                 dtype=np.float32)
        in_maps.append(
            {"S": S128, "U": U, "BDS": BDS, "BDM": BDM, "B2S": B2S})
    return in_maps


def _gather_output(results, inputs):
    """Combine per-core OUTM/OUTS into the full [B, N, CO] output."""
    b2 = np.asarray(inputs["b2"], np.float32)
    out = np.zeros((B, N, CO), np.float32)
    for core, res in enumerate(results):
        b = core // CORES_PER_B
        n0 = (core % CORES_PER_B) * N_PER_CORE
        # partition p = 64*half + 8*gp + j ; columns = groups g
        m = np.asarray(res["OUTM"], np.float32).reshape(2, 8, 8, NG)
        s = np.asarray(res["OUTS"], np.float32).reshape(2, 8, 8, NG)
        m = m.max(axis=0)          # [gp, j, g] max over chunk-parity halves
        s = s.sum(axis=0)          # [gp, j, g] sum over halves
        m = m.transpose(2, 0, 1).reshape(N_PER_CORE, 8)   # [n-n0, j]
        s = s.transpose(2, 0, 1).reshape(N_PER_CORE, 8)
        out[b, n0:n0 + N_PER_CORE, :CO // 2] = np.maximum(
            m + b2[:CO // 2], 0.0)
        out[b, n0:n0 + N_PER_CORE, CO // 2:] = s / float(PAIRS)
    return out


def _run(inputs, trace=False, **spmd_kwargs):
    if "nc" not in _CACHE:
        _CACHE["nc"] = _build_bass()
    nc = _CACHE["nc"]
    in_maps = _prep_inputs(inputs)
    res = bass_utils.run_bass_kernel_spmd(
        nc, in_maps, core_ids=list(range(NCORES)), trace=trace, **spmd_kwargs)
    return _gather_output(res.results, inputs), res


def kernel(**inputs):
    out, _ = _run(inputs)
    return out


# revision 9
# speedup vs baseline: 1.1813x; 1.1323x over previous
"""Trainium2 Bass kernel for nn_PointPairwiseRelation3.

Reference computation (B=2, N=256, N1=N2=128, C=Co=16):
    h1[b,n,j,k,:] = relu(x[b,n]@Wa + (x1[b,j]-x[b,n])@Wb + (x2[b,k]-x[b,n])@Wc + b1)
    h2 = relu(h1 @ W2 + b2)
    out[b,n,:8]  = max_{j,k}  h2[..., :8]
    out[b,n,8:]  = mean_{j,k} h2[..., 8:]

Key decomposition: the first-layer pre-activation is u[b,n] + p[b,j] + q[b,k]
with u = x@(Wa-Wb-Wc)+b1, p = x1@Wb, q = x2@Wc (tiny host-side matmuls).
S[b] = p (+) q (outer sum over all (j,k) pairs) is shared by every query
point n of batch b; it is precomputed on the host, replicated over the 8
partition groups, and DMA'd in.

Device layout (per core; 8 cores, each handles 64 query points of one b):
  - partitions of T = 8 query points (gp) x 16 channels (c): part = 16*gp+c
  - free dim = the 16384 (j,k) pairs, in 2048-wide T tiles (4 chunks of 512)
  - 8 groups g of 8 query points each.

Channel-split second layer (mean-pooled channels are o>=8, max-pooled o<8):
two M=64 stationary matrices BDS = kron(I8, W2[:, 8:]) and
BDM = kron(I8, W2[:, :8]). For each pair of 512-col chunks (A, B):
    PSUM_s[ 0: 64, fd] = BDS.T @ T_A     PSUM_m[ 0: 64, fd] = BDM.T @ T_A
    PSUM_s[64:128, fd] = BDS.T @ T_B     PSUM_m[64:128, fd] = BDM.T @ T_B
so one [128, 1024] PSUM tile carries TWO chunks (2048 cols) of sum-half
(or max-half) data -> each pooling pass touches half the elements:
  ACT : activation(relu, PSUM_s + b2s, accum_out=sums slot)   (mean pool)
  DVE : tensor_tensor_reduce(max over PSUM_m, chained via scalar init)
        (max pool, fp32; relu+b2 deferred to the host: both monotonic)
  DVE : T = relu(S + u_g) via tensor_scalar (bf16 4x mode); N_ACT_T of the
        8 per-group T tiles are produced on ACT (activation bias=u) to
        balance engine load.
Host gather: mean = (sum_halfA + sum_halfB)/16384, max =
relu(max(max_halfA, max_halfB) + b2).
"""

import numpy as np
import ml_dtypes

import concourse.bass as bass
import concourse.bacc as bacc
import concourse.mybir as mybir
from concourse.tile import TileContext
from concourse import bass_utils

# Problem constants (hardcoded; kernel.py must be self-contained).
B = 2
N = 256
N1 = 128
N2 = 128
C = 16
CO = 16
NCORES = 8
CORES_PER_B = NCORES // B          # 4 cores per batch element
N_PER_CORE = N // CORES_PER_B      # 64 query points per core
NG = N_PER_CORE // 8               # 8 groups of 8 query points
PAIRS = N1 * N2                    # 16384
TCH = 2048                         # T tile width (4 chunks of 512)
NT = PAIRS // TCH                  # 8 T tiles per group
PFD = TCH // 2                     # PSUM tile free dim (2 chunks stacked)
ACT_T_SLOTS = (4,)                 # which T-tiles ACT produces (mid-group)

BF16 = mybir.dt.bfloat16
F32 = mybir.dt.float32

_CACHE = {}


def _build_bass():
    nc = bacc.Bacc("TRN2", target_bir_lowering=False)

    S_d = nc.dram_tensor("S", (128, PAIRS), BF16, kind="ExternalInput")
    U_d = nc.dram_tensor("U", (128, NG), F32, kind="ExternalInput")
    BDS_d = nc.dram_tensor("BDS", (128, 64), BF16, kind="ExternalInput")
    BDM_d = nc.dram_tensor("BDM", (128, 64), BF16, kind="ExternalInput")
    B2S_d = nc.dram_tensor("B2S", (128, 1), F32, kind="ExternalInput")
    OUTS_d = nc.dram_tensor("OUTS", (128, NG), F32, kind="ExternalOutput")
    OUTM_d = nc.dram_tensor("OUTM", (128, NG), F32, kind="ExternalOutput")

    add = mybir.AluOpType.add
    amax = mybir.AluOpType.max
    relu = mybir.ActivationFunctionType.Relu
    AX = mybir.AxisListType.X

    with TileContext(nc) as tc:
        with (
            tc.tile_pool(name="const", bufs=1) as cpool,
            tc.tile_pool(name="work", bufs=3) as wpool,
            tc.tile_pool(name="psum", bufs=2, space="PSUM") as ppool,
            tc.tile_pool(name="red", bufs=2) as rpool,
        ):
            # Constants / inputs. S is loaded in TCH-wide sections so the
            # first groups' compute can start before the whole 4MB arrives.
            S_sec = []
            for s in range(NT):
                t = cpool.tile([128, TCH], BF16, tag=f"S{s}")
                nc.sync.dma_start(out=t, in_=S_d[:, s * TCH:(s + 1) * TCH])
                S_sec.append(t)
            U_t = cpool.tile([128, NG], F32, tag="U")
            nc.sync.dma_start(out=U_t, in_=U_d[:, :])
            BDS_t = cpool.tile([128, 64], BF16, tag="BDS")
            nc.sync.dma_start(out=BDS_t, in_=BDS_d[:, :])
            BDM_t = cpool.tile([128, 64], BF16, tag="BDM")
            nc.sync.dma_start(out=BDM_t, in_=BDM_d[:, :])
            B2S_t = cpool.tile([128, 1], F32, tag="B2S")
            nc.sync.dma_start(out=B2S_t, in_=B2S_d[:, :])
            OUTS_t = cpool.tile([128, NG], F32, tag="OUTS")
            OUTM_t = cpool.tile([128, NG], F32, tag="OUTM")

            for g in range(NG):
                sums = rpool.tile([128, NT], F32, tag="sums")
                mslots = rpool.tile([128, NT], F32, tag="mslots")
                for t in range(NT):
                    T_t = wpool.tile([128, TCH], BF16, tag="T")
                    if t in ACT_T_SLOTS:
                        # ACT-produced T: relu(1.0 * S + u) via activation.
                        nc.scalar.activation(
                            out=T_t, in_=S_sec[t], func=relu,
                            bias=U_t[:, g:g + 1],
                        )
                    else:
                        nc.vector.tensor_scalar(
                            out=T_t, in0=S_sec[t],
                            scalar1=U_t[:, g:g + 1], scalar2=0.0,
                            op0=add, op1=amax,
                        )
                    Ms = ppool.tile([128, PFD], F32, tag="Ms")
                    Mm = ppool.tile([128, PFD], F32, tag="Mm")
                    for pr in range(TCH // 1024):
                        fa = 1024 * pr
                        fb = fa + 512
                        fd = 512 * pr
                        nc.tensor.matmul(
                            Ms[0:64, fd:fd + 512], BDS_t,
                            T_t[:, fa:fa + 512], start=True, stop=True)
                        nc.tensor.matmul(
                            Ms[64:128, fd:fd + 512], BDS_t,
                            T_t[:, fb:fb + 512], start=True, stop=True)
                        nc.tensor.matmul(
                            Mm[0:64, fd:fd + 512], BDM_t,
                            T_t[:, fa:fa + 512], start=True, stop=True)
                        nc.tensor.matmul(
                            Mm[64:128, fd:fd + 512], BDM_t,
                            T_t[:, fb:fb + 512], start=True, stop=True)
                    # Mean-pool half: relu + bias + accumulate on ACT.
                    Js = wpool.tile([128, PFD], BF16, tag="Js")
                    nc.scalar.activation(
                        out=Js, in_=Ms, func=relu, bias=B2S_t[:, 0:1],
                        accum_out=sums[:, t:t + 1],
                    )
                    # Max-pool half: per-tile max-reduce off PSUM (fp32).
                    nc.vector.tensor_reduce(
                        out=mslots[:, t:t + 1], in_=Mm, axis=AX, op=amax)
                nc.vector.tensor_reduce(
                    out=OUTS_t[:, g:g + 1], in_=sums, axis=AX, op=add)
                nc.vector.tensor_reduce(
                    out=OUTM_t[:, g:g + 1], in_=mslots, axis=AX, op=amax)

            nc.sync.dma_start(out=OUTS_d[:, :], in_=OUTS_t)
            nc.sync.dma_start(out=OUTM_d[:, :], in_=OUTM_t)

    nc.finalize()
    return nc


def _prep_inputs(inputs):
    """Host-side prep: tiny projections + per-core input maps."""
    x = np.asarray(inputs["x"], np.float32)
    x1 = np.asarray(inputs["x1"], np.float32)
    x2 = np.asarray(inputs["x2"], np.float32)
    W1 = np.asarray(inputs["W1"], np.float32)
    b1 = np.asarray(inputs["b1"], np.float32)
    W2 = np.asarray(inputs["W2"], np.float32)
    b2 = np.asarray(inputs["b2"], np.float32)

    Wa, Wb, Wc = W1[:C], W1[C:2 * C], W1[2 * C:]
    u = x @ (Wa - Wb - Wc) + b1          # [B, N, CO]
    p = x1 @ Wb                          # [B, N1, CO]
    q = x2 @ Wc                          # [B, N2, CO]

    eye8 = np.eye(8, dtype=np.float32)
    BDS = np.kron(eye8, W2[:, CO // 2:]).astype(ml_dtypes.bfloat16)
    BDM = np.kron(eye8, W2[:, :CO // 2]).astype(ml_dtypes.bfloat16)
    B2S = np.tile(b2[CO // 2:], 16).reshape(128, 1).astype(np.float32)

    in_maps = []
    for core in range(NCORES):
        b = core // CORES_PER_B
        n0 = (core % CORES_PER_B) * N_PER_CORE
        # S16[c, j*N2+k] = p[b,j,c] + q[b,k,c]; replicate over 8 groups.
        S16 = (p[b][:, None, :] + q[b][None, :, :]).transpose(2, 0, 1)
        S16 = S16.reshape(CO, PAIRS)
        S128 = np.tile(S16, (8, 1)).astype(ml_dtypes.bfloat16)
        # U[16*gp+c, g] = u[b, n0+8*g+gp, c]
        U = u[b, n0:n0 + N_PER_CORE].reshape(NG, 8, CO)
        U = np.ascontiguousarray(U.transpose(1, 2, 0).reshape(128, NG),
                                 dtype=np.float32)
        in_maps.append(
            {"S": S128, "U": U, "BDS": BDS, "BDM": BDM, "B2S": B2S})
    return in_maps


def _gather_output(results, inputs):
    """Combine per-core OUTM/OUTS into the full [B, N, CO] output."""
    b2 = np.asarray(inputs["b2"], np.float32)
    out = np.zeros((B, N, CO), np.float32)
    for core, res in enumerate(results):
        b = core // CORES_PER_B
        n0 = (core % CORES_PER_B) * N_PER_CORE
        # partition p = 64*half + 8*gp + j ; columns = groups g
        m = np.asarray(res["OUTM"], np.float32).reshape(2, 8, 8, NG)
        s = np.asarray(res["OUTS"], np.float32).reshape(2, 8, 8, NG)
        m = m.max(axis=0)          # [gp, j, g] max over chunk-parity halves
        s = s.sum(axis=0)          # [gp, j, g] sum over halves
        m = m.transpose(2, 0, 1).reshape(N_PER_CORE, 8)   # [n-n0, j]
        s = s.transpose(2, 0, 1).reshape(N_PER_CORE, 8)
        out[b, n0:n0 + N_PER_CORE, :CO // 2] = np.maximum(
            m + b2[:CO // 2], 0.0)
        out[b, n0:n0 + N_PER_CORE, CO // 2:] = s / float(PAIRS)
    return out


def _run(inputs, trace=False, **spmd_kwargs):
    if "nc" not in _CACHE:
        _CACHE["nc"] = _build_bass()
    nc = _CACHE["nc"]
    in_maps = _prep_inputs(inputs)
    res = bass_utils.run_bass_kernel_spmd(
        nc, in_maps, core_ids=list(range(NCORES)), trace=trace, **spmd_kwargs)
    return _gather_output(res.results, inputs), res


def kernel(**inputs):
    out, _ = _run(inputs)
    return out
